# revision 3
# baseline (speedup 1.0000x reference)
"""Grouped GEMM (MoE expert-parallel) Trainium2 kernel.

Problem: inp [16384, 4096] f32, weight [8, 4096, 4096] f32 ([e, out_f, in_d]),
tokens pre-grouped by expert, 2048 tokens/expert.
out[e*2048+m, f] = sum_d inp[e*2048+m, d] * weight[e, f, d].

Strategy: expert-parallel, one expert per NeuronCore (8 cores). No collectives.
Host-side layout: each core receives xt = x_e^T [D, M] and wt = w_e^T [D, F]
(both d-major, so the contraction dim lands on SBUF partitions with natural
DMAs). The device computes outT = w_e @ x_e^T as [F, M] with the weight tile
stationary ([128d, 128f]) and activations moving ([128d, 512m]); the host
transposes outT back to [M, F] while gathering.

Per-core roofline: 2048*4096*4096 MACs / (128*128 @ 2.4GHz) = ~873 us PE,
~190 MB DMA (~530 us) with the m-halved weight-streaming schedule below.
Matmuls run as float32r (full-rate fp32 streaming mode, 1 cycle/row for
moving dims >= 256).
"""

import numpy as np

E = 8
M = 2048  # tokens per expert
D = 4096  # in features (contraction)
F = 4096  # out features
P = 128

KO = D // P  # 32 k-subtiles
FO = F // P  # 32 f blocks (stationary tiles per k)
MSEG = 512  # moving free dim per matmul
MH = 1024  # m half resident in the schedule

_cache = {}


def _build_nc(dtype_tag="f32r"):
    import concourse.bass as bass
    import concourse.mybir as mybir
    import concourse.tile as tile
    from concourse import bacc

    f32 = mybir.dt.float32
    mm_dt = mybir.dt.float32r if dtype_tag == "f32r" else mybir.dt.float32

    nc = bacc.Bacc(None, target_bir_lowering=False, debug=False)

    xt_d = nc.dram_tensor("xt", [D, M], f32, kind="ExternalInput")
    wt_d = nc.dram_tensor("wt", [D, F], f32, kind="ExternalInput")
    ot_d = nc.dram_tensor("ot", [F, M], f32, kind="ExternalOutput")

    # d-major -> partition-major views
    xt_r = xt_d[:].rearrange("(ko p) m -> p ko m", p=P)  # [128, 32, 2048]
    wt_r = wt_d[:].rearrange("(ko p) f -> p ko f", p=P)  # [128, 32, 4096]
    ot_r = ot_d[:].rearrange("(fo p) m -> p fo m", p=P)  # [128, 32, 2048]

    n_mh = M // MH
    n_seg = MH // MSEG  # moving segments per m-half

    with tile.TileContext(nc) as tc:
        with (
            tc.tile_pool(name="xres", bufs=1) as xres,
            tc.tile_pool(name="wstream", bufs=3) as wstream,
            tc.tile_pool(name="evict", bufs=3) as evict,
            tc.tile_pool(name="psum", bufs=8, space="PSUM") as psum,
        ):
            for mh in range(n_mh):
                m0 = mh * MH
                # resident activation half [128, 32, MH] = 16MB, split DMAs so
                # the first f-block can start before the whole half lands
                xt_sb = xres.tile([P, KO, MH], mm_dt, tag="xres")
                for j in range(n_seg):
                    nc.sync.dma_start(
                        xt_sb[:, :, j * MSEG : (j + 1) * MSEG],
                        xt_r[:, :, m0 + j * MSEG : m0 + (j + 1) * MSEG].bitcast(mm_dt),
                    )

                fo_range = range(FO) if mh % 2 == 0 else range(FO - 1, -1, -1)
                for fo in fo_range:
                    # stationary weights for this f block, all k: [128, 32, 128]
                    wt_sb = wstream.tile([P, KO, P], mm_dt, tag="w")
                    nc.sync.dma_start(
                        wt_sb[:], wt_r[:, :, fo * P : (fo + 1) * P].bitcast(mm_dt)
                    )

                    ps = [
                        psum.tile([P, MSEG], f32, tag="acc", name=f"ps_{mh}_{fo}_{s}")
                        for s in range(n_seg)
                    ]
                    for k in range(KO):
                        lhsT = wt_sb[:, k, :]
                        for s in range(n_seg):
                            nc.tensor.matmul(
                                ps[s],
                                lhsT,
                                xt_sb[:, k, s * MSEG : (s + 1) * MSEG],
                                start=(k == 0),
                                stop=(k == KO - 1),
                            )
                    ot_sb = evict.tile([P, MH], f32, tag="ev")
                    for s in range(n_seg):
                        nc.vector.tensor_copy(
                            ot_sb[:, s * MSEG : (s + 1) * MSEG], ps[s]
                        )
                    nc.sync.dma_start(
                        ot_r[:, fo, m0 : m0 + MH], ot_sb[:]
                    )

    nc.compile()
    return nc


def _get_nc():
    if "nc" not in _cache:
        _cache["nc"] = _build_nc()
    return _cache["nc"]


def kernel(inp, weight, num_tokens_per_expert):
    from concourse.bass_utils import run_bass_kernel_spmd

    inp = np.asarray(inp)
    weight = np.asarray(weight)
    assert inp.shape == (E * M, D) and weight.shape == (E, F, D)

    nc = _get_nc()
    in_maps = [
        {
            "xt": np.ascontiguousarray(inp[e * M : (e + 1) * M].T),
            "wt": np.ascontiguousarray(weight[e].T),
        }
        for e in range(E)
    ]
    res = run_bass_kernel_spmd(nc, in_maps, list(range(E)))
    out = np.empty((E * M, F), dtype=np.float32)
    for e in range(E):
        out[e * M : (e + 1) * M] = res.results[e]["ot"].T
    return out


# revision 4
# speedup vs baseline: 1.0573x; 1.0573x over previous
"""Grouped GEMM (MoE expert-parallel) Trainium2 kernel.

Problem: inp [16384, 4096] f32, weight [8, 4096, 4096] f32 ([e, out_f, in_d]),
tokens pre-grouped by expert, 2048 tokens/expert.
out[e*2048+m, f] = sum_d inp[e*2048+m, d] * weight[e, f, d].

Strategy: expert-parallel, one expert per NeuronCore (8 cores), no
collectives. Host-side layout: each core receives xt = x_e^T [D, M] and
wt = w_e^T [D, F] (both d-major so the contraction dim lands on SBUF
partitions with natural DMAs). The device computes outT = w_e @ x_e^T as
[F, M] with the weight tile stationary ([128d, 128f]) and activations
moving ([128d, 512m]); the host transposes outT back while gathering.

Schedule: split-K zigzag over (kh, mh) blocks - xt resident blocks are
[128, 16, 1024] (8MB) in a 2-slot ring so the next block always prefetches
behind the current one (no pipeline bubble, which cost ~60us in the
full-K variant). kh=0 partials go to a DRAM scratch tensor; kh=1 combines
them during PSUM eviction with a vector add.

Matmuls run as float32r (full-rate fp32 streaming mode: 1 cycle/row vs 4
for strict fp32; ~tf32 mantissa, measured rel err ~1.5e-4).
Per-core: 4096 matmuls ([128k,128f] x [128k,512m]) ~= 980us PE-bound.
"""

import numpy as np

E = 8
M = 2048  # tokens per expert
D = 4096  # in features (contraction)
F = 4096  # out features
P = 128

KO = D // P  # 32 k-subtiles
FO = F // P  # 32 f blocks (stationary tiles per k)
MSEG = 512  # moving free dim per matmul
KH = 2  # split-K passes
KC = KO // KH  # 16 k-subtiles per pass
MB = 1024  # m block resident in SBUF
NMB = M // MB  # 2

_cache = {}


def _build_nc(dtype_tag="f32r"):
    import concourse.bass as bass
    import concourse.mybir as mybir
    import concourse.tile as tile
    from concourse import bacc

    f32 = mybir.dt.float32
    mm_dt = mybir.dt.float32r if dtype_tag == "f32r" else mybir.dt.float32

    nc = bacc.Bacc(None, target_bir_lowering=False, debug=False)

    xt_d = nc.dram_tensor("xt", [D, M], f32, kind="ExternalInput")
    wt_d = nc.dram_tensor("wt", [D, F], f32, kind="ExternalInput")
    ot_d = nc.dram_tensor("ot", [F, M], f32, kind="ExternalOutput")
    pt_d = nc.dram_tensor("ptmp", [F, M], f32)  # kh=0 partial sums

    # d-major -> partition-major views
    xt_r = xt_d[:].rearrange("(ko p) m -> p ko m", p=P)  # [128, 32, 2048]
    wt_r = wt_d[:].rearrange("(ko p) f -> p ko f", p=P)  # [128, 32, 4096]
    ot_r = ot_d[:].rearrange("(fo p) m -> p fo m", p=P)  # [128, 32, 2048]
    pt_r = pt_d[:].rearrange("(fo p) m -> p fo m", p=P)

    n_seg = MB // MSEG  # 2 moving segments per m block

    # zigzag so consecutive blocks differ in exactly one coordinate and the
    # xt ring (bufs=2) always prefetches the next block during the current
    blocks = [(0, 0), (0, 1), (1, 1), (1, 0)]

    with tile.TileContext(nc) as tc:
        with (
            tc.tile_pool(name="xblk", bufs=2) as xblk,
            tc.tile_pool(name="wstream", bufs=3) as wstream,
            tc.tile_pool(name="pin", bufs=3) as pin,
            tc.tile_pool(name="evict", bufs=3) as evict,
            tc.tile_pool(name="psum", bufs=8, space="PSUM") as psum,
        ):
            for bi, (kh, mh) in enumerate(blocks):
                m0 = mh * MB
                k0 = kh * KC
                # resident activation block [128, 16, 1024] = 8MB, one DMA
                # per k-subtile so the first matmul only waits for ~512KB
                xt_sb = xblk.tile([P, KC, MB], mm_dt, tag="x")
                for kc in range(KC):
                    nc.sync.dma_start(
                        xt_sb[:, kc, :],
                        xt_r[:, k0 + kc, m0 : m0 + MB].bitcast(mm_dt),
                    )

                fo_range = range(FO) if bi % 2 == 0 else range(FO - 1, -1, -1)
                for fo in fo_range:
                    # stationary weights for this f block, kh's k range
                    wt_sb = wstream.tile([P, KC, P], mm_dt, tag="w")
                    nc.sync.dma_start(
                        wt_sb[:],
                        wt_r[:, k0 : k0 + KC, fo * P : (fo + 1) * P].bitcast(mm_dt),
                    )

                    ps = [
                        psum.tile([P, MSEG], f32, tag="acc", name=f"ps_{bi}_{fo}_{s}")
                        for s in range(n_seg)
                    ]
                    for k in range(KC):
                        for s in range(n_seg):
                            nc.tensor.matmul(
                                ps[s],
                                wt_sb[:, k, :],
                                xt_sb[:, k, s * MSEG : (s + 1) * MSEG],
                                start=(k == 0),
                                stop=(k == KC - 1),
                            )

                    ot_sb = evict.tile([P, MB], f32, tag="ev")
                    if kh == 0:
                        for s in range(n_seg):
                            nc.vector.tensor_copy(
                                ot_sb[:, s * MSEG : (s + 1) * MSEG], ps[s]
                            )
                        nc.sync.dma_start(pt_r[:, fo, m0 : m0 + MB], ot_sb[:])
                    else:
                        pin_sb = pin.tile([P, MB], f32, tag="pi")
                        nc.sync.dma_start(pin_sb[:], pt_r[:, fo, m0 : m0 + MB])
                        for s in range(n_seg):
                            nc.vector.tensor_tensor(
                                ot_sb[:, s * MSEG : (s + 1) * MSEG],
                                pin_sb[:, s * MSEG : (s + 1) * MSEG],
                                ps[s],
                                mybir.AluOpType.add,
                            )
                        nc.sync.dma_start(ot_r[:, fo, m0 : m0 + MB], ot_sb[:])

    nc.compile()
    return nc


def _get_nc():
    if "nc" not in _cache:
        _cache["nc"] = _build_nc()
    return _cache["nc"]


def kernel(inp, weight, num_tokens_per_expert):
    from concourse.bass_utils import run_bass_kernel_spmd

    inp = np.asarray(inp)
    weight = np.asarray(weight)
    assert inp.shape == (E * M, D) and weight.shape == (E, F, D)

    nc = _get_nc()
    in_maps = [
        {
            "xt": np.ascontiguousarray(inp[e * M : (e + 1) * M].T),
            "wt": np.ascontiguousarray(weight[e].T),
        }
        for e in range(E)
    ]
    res = run_bass_kernel_spmd(nc, in_maps, list(range(E)))
    out = np.empty((E * M, F), dtype=np.float32)
    for e in range(E):
        out[e * M : (e + 1) * M] = res.results[e]["ot"].T
    return out
